# revision 7
# baseline (speedup 1.0000x reference)
"""Trainium2 Bass kernel for nn_CausalFullAttention (8 NeuronCores, SPMD).

Sharding: head-parallel — core h owns head h end-to-end (projections, decay
scan, causal attention), then an on-device AllGather of the per-head output
(transposed layout) lets every core compute a 128-column slice of the final
to_out projection. Host only slices weights / transposes x (layout prep) and
concatenates the 8 output slices.

Numerics: f32r (11-bit mantissa fast matmul) for qkv projections, attention
and to_out; full fp32 for the a-projection and the cumsum tri-matmuls (the
decay scan amplifies rounding); bf16 square trick for the RMS norm row sums.
Host-emulated end-to-end error vs the fp32 reference: ~3.7e-4 Frobenius.
"""
import sys

for _p in ("/opt/trn_rl_repo", "/opt/pypackages"):
    if _p not in sys.path:
        sys.path.append(_p)

import numpy as np
import concourse.bass as bass
import concourse.mybir as mybir
from concourse import bacc, tile
from concourse.bass_utils import run_bass_kernel_spmd

F32 = mybir.dt.float32
F32R = mybir.dt.float32r
BF16 = mybir.dt.bfloat16
I32 = mybir.dt.int32
AF = mybir.ActivationFunctionType
ALU = mybir.AluOpType

HEADS = 8
DH = 64
SEQ = 4096
DIM = 1024
DI = 512               # DIM_INNER
SCALE = DH ** -0.5
P = 128
NT = SEQ // P          # 32 row tiles
NPAN = 8               # q/row panels of 512
PW = 512               # panel width (rows)
NC_ = DIM // P         # 8 contraction chunks
PI = float(np.pi)

_cache = {}


def _build():
    nc = bacc.Bacc("TRN2", target_bir_lowering=False, debug=False,
                   enable_asserts=True, num_devices=8)

    din = {}
    for name, shp in [("xT", [DIM, SEQ]), ("Wqk", [DIM, 128]), ("Wv", [DIM, DH]),
                      ("Wa", [DIM, 128]), ("Wo", [DI, 128]), ("bo", [P, 1]),
                      ("ident", [P, P]), ("Utri", [P, P]), ("maskP", [4 * P, PW])]:
        din[name] = nc.dram_tensor(name, shp, F32, kind="ExternalInput").ap()
    dout = nc.dram_tensor("out", [P, SEQ], F32, kind="ExternalOutput").ap()
    dbg = {}
    if _cache.get("debug"):
        for nm, shp in [("dbg_qkT", [P, SEQ]), ("dbg_a", [P, SEQ]),
                        ("dbg_y", [P, SEQ]), ("dbg_cum", [P, SEQ]),
                        ("dbg_A", [P, NT * DH]), ("dbg_Ainv", [P, NT * DH]),
                        ("dbg_qT", [DH, SEQ]), ("dbg_kT", [DH, SEQ]),
                        ("dbg_v", [P, NT * DH]), ("dbg_s", [P, NT]),
                        ("dbg_outT", [DH, SEQ])]:
            dbg[nm] = nc.dram_tensor(nm, shp, F32, kind="ExternalOutput").ap()

    with tile.TileContext(nc) as tc:
        with tc.tile_pool(name="wt", bufs=1) as wt, \
             tc.tile_pool(name="big", bufs=1) as bg, \
             tc.tile_pool(name="io", bufs=1) as io, \
             tc.tile_pool(name="ps", bufs=1, space="PSUM") as ps, \
             tc.tile_pool(name="dr", bufs=1, space="DRAM") as dr:

            # ---------------- weights / constants ----------------
            Wqk_r, Wv_r, Wa_f = [], [], []
            for c in range(NC_):
                w1 = wt.tile([P, 128], F32R, name=f"wqk{c}", tag=f"wqk{c}")
                nc.gpsimd.dma_start(w1[:], din["Wqk"][c * P:(c + 1) * P, :])
                Wqk_r.append(w1)
                w2 = wt.tile([P, DH], F32R, name=f"wv{c}", tag=f"wv{c}")
                nc.gpsimd.dma_start(w2[:], din["Wv"][c * P:(c + 1) * P, :])
                Wv_r.append(w2)
                w3 = wt.tile([P, 128], F32, name=f"wa{c}", tag=f"wa{c}")
                nc.sync.dma_start(w3[:], din["Wa"][c * P:(c + 1) * P, :])
                Wa_f.append(w3)
            Wo_r = []
            for c in range(4):
                w4 = wt.tile([P, 128], F32R, name=f"wo{c}", tag=f"wo{c}")
                nc.gpsimd.dma_start(w4[:], din["Wo"][c * P:(c + 1) * P, :])
                Wo_r.append(w4)
            bo = wt.tile([P, 1], F32, name="bo", tag="bo")
            nc.sync.dma_start(bo[:], din["bo"][:])
            ident = wt.tile([P, P], F32, name="ident", tag="ident")
            nc.sync.dma_start(ident[:], din["ident"][:])
            Utri = wt.tile([P, P], F32, name="Utri", tag="Utri")
            nc.sync.dma_start(Utri[:], din["Utri"][:])
            maskP = []
            for m in range(4):
                mk = wt.tile([P, PW], F32, name=f"maskP{m}", tag=f"maskP{m}")
                nc.sync.dma_start(mk[:], din["maskP"][m * P:(m + 1) * P, :])
                maskP.append(mk)
            ones_row = wt.tile([1, P], F32, name="ones_row", tag="ones_row")
            nc.vector.memset(ones_row[:], 1.0)
            ones_bf = wt.tile([P, 1], BF16, name="ones_bf", tag="ones_bf")
            nc.vector.memset(ones_bf[:], 1.0)
            halfpi = wt.tile([P, 1], F32, name="halfpi", tag="halfpi")
            nc.vector.memset(halfpi[:], PI / 2)

            # ---------------- persistent big tensors ----------------
            qkT = bg.tile([P, SEQ], F32, name="qkT", tag="qkT")         # q rows 0-63, k 64-127
            v_all = bg.tile([P, NT * DH], F32R, name="v_all", tag="v_all")
            a_sc = bg.tile([P, SEQ], F32, name="a_sc", tag="a_sc")      # a (scaled); later reused for ABT
            y_full = bg.tile([P, SEQ], F32, name="y_full", tag="y_full")  # [sp|th] per tile; later cum
            mag_full = bg.tile([P, NT * DH], F32, name="mag_full", tag="mag_full")
            d1 = bg.tile([P, NT * DH], F32, name="d1", tag="d1")
            d2 = bg.tile([P, NT * DH], F32, name="d2", tag="d2")
            A_full = bg.tile([P, NT * DH], F32, name="A_full", tag="A_full")
            qT_eff = bg.tile([DH, SEQ], F32R, name="qT_eff", tag="qT_eff")
            kT_eff = bg.tile([DH, SEQ], F32R, name="kT_eff", tag="kT_eff")
            sT_all = bg.tile([P, NT], F32, name="sT_all", tag="sT_all")
            nrm_all = bg.tile([P, NT], F32, name="nrm_all", tag="nrm_all")
            s_all = bg.tile([P, NT], F32, name="s_all", tag="s_all")

            scratch = dr.tile([NPAN, PW], F32, name="scratch", tag="scratch")
            cc_in = dr.tile([DH, SEQ], F32, name="cc_in", tag="cc_in")
            cc_out = dr.tile([DI, SEQ], F32, name="cc_out", tag="cc_out",
                             addr_space="Shared")

            # ================= Phase A: projections + norm =================
            for p in range(NPAN):
                xt, xtr = [], []
                for c in range(NC_):
                    t_ = io.tile([P, PW], F32, name=f"xt_{p}_{c}", tag="xt", bufs=9)
                    nc.sync.dma_start(t_[:], din["xT"][c * P:(c + 1) * P,
                                                       p * PW:(p + 1) * PW])
                    xt.append(t_)
                    tr = io.tile([P, PW], F32R, name=f"xtr_{p}_{c}", tag="xtr", bufs=9)
                    nc.gpsimd.dma_start(tr[:], t_[:])
                    xtr.append(tr)

                # norm row sums via bf16 squares
                ss_ps = ps.tile([1, PW], F32, name=f"ss_{p}", tag="ss", bufs=1)
                for c in range(NC_):
                    sq = io.tile([P, PW], BF16, name=f"sq_{p}_{c}", tag="sq", bufs=3)
                    if c % 2 == 0:
                        nc.scalar.activation(sq[:], xt[c][:], AF.Square)
                    else:
                        nc.vector.tensor_tensor(sq[:], xt[c][:], xt[c][:], ALU.mult)
                    nc.tensor.matmul(ss_ps[:], ones_bf[:], sq[:],
                                     start=(c == 0), stop=(c == NC_ - 1))
                ss_sb = io.tile([1, PW], F32, name=f"ssb_{p}", tag="ssb", bufs=2)
                nc.vector.tensor_copy(ss_sb[:], ss_ps[:])
                nc.sync.dma_start(scratch[p:p + 1, :], ss_sb[:])
                sT_p = sT_all[:, p * 4:(p + 1) * 4]
                nc.sync.dma_start(
                    sT_p, scratch[p:p + 1, :].rearrange("o (t r) -> r (o t)", t=4))
                nrm_p = nrm_all[:, p * 4:(p + 1) * 4]
                nc.scalar.activation(nrm_p, sT_p, AF.Sqrt)
                rcp_p = s_all[:, p * 4:(p + 1) * 4]
                nc.vector.reciprocal(rcp_p, nrm_p)
                nc.vector.tensor_scalar(rcp_p, rcp_p, 32.0, None, op0=ALU.mult)

                # qkT (f32r)
                qk_ps = ps.tile([P, PW], F32, name=f"qk_{p}", tag="mm", bufs=5)
                for c in range(NC_):
                    nc.tensor.matmul(qk_ps[:], Wqk_r[c][:], xtr[c][:],
                                     start=(c == 0), stop=(c == NC_ - 1))
                nc.scalar.copy(qkT[:, p * PW:(p + 1) * PW], qk_ps[:])

                # vT (f32r) then transpose to row layout with s-scale
                v_ps = ps.tile([DH, PW], F32, name=f"v_{p}", tag="mm", bufs=5)
                for c in range(NC_):
                    nc.tensor.matmul(v_ps[:], Wv_r[c][:], xtr[c][:],
                                     start=(c == 0), stop=(c == NC_ - 1))
                vT_sb = io.tile([DH, PW], F32, name=f"vts_{p}", tag="vt", bufs=2)
                nc.scalar.copy(vT_sb[:], v_ps[:])
                for tt in range(4):
                    g = p * 4 + tt
                    vp2 = ps.tile([P, DH], F32, name=f"vp2_{p}_{tt}", tag="mm", bufs=5)
                    nc.tensor.transpose(vp2[:], vT_sb[:, tt * P:(tt + 1) * P],
                                        ident[0:DH, 0:DH])
                    nc.vector.tensor_scalar(v_all[:, g * DH:(g + 1) * DH], vp2[:],
                                            s_all[:, g:g + 1], None, op0=ALU.mult)

                # a projection (fp32) + row scale
                for tt in range(4):
                    g = p * 4 + tt
                    a_ps = ps.tile([P, 128], F32, name=f"a_{p}_{tt}", tag="mm", bufs=5)
                    for c in range(NC_):
                        nc.tensor.matmul(a_ps[:], xt[c][:, tt * P:(tt + 1) * P],
                                         Wa_f[c][:], start=(c == 0),
                                         stop=(c == NC_ - 1))
                    nc.vector.tensor_scalar(a_sc[:, g * P:(g + 1) * P], a_ps[:],
                                            s_all[:, g:g + 1], None, op0=ALU.mult)

            if dbg:
                nc.sync.dma_start(dbg["dbg_qkT"][:], qkT[:])
                nc.sync.dma_start(dbg["dbg_a"][:], a_sc[:])
                nc.sync.dma_start(dbg["dbg_s"][:], s_all[:])

            # ================= Phase B: decay elementwise =================
            # a_sc per tile: cols g*128+(2d+c); re: c=0, im: c=1
            re_ap = a_sc[:].rearrange("p (t d c) -> p (t d) c", c=2, d=DH)[:, :, 0]
            im_ap = a_sc[:].rearrange("p (t d c) -> p (t d) c", c=2, d=DH)[:, :, 1]
            sp_out = y_full[:].rearrange("p (t q d) -> p t q d", q=2, d=DH)[:, :, 0, :]
            th_out = y_full[:].rearrange("p (t q d) -> p t q d", q=2, d=DH)[:, :, 1, :]
            nc.vector.tensor_tensor(d1[:], re_ap, re_ap, ALU.mult)
            nc.vector.tensor_tensor(d2[:], im_ap, im_ap, ALU.mult)
            nc.vector.tensor_tensor(mag_full[:], d1[:], d2[:], ALU.add)
            nc.scalar.activation(d1[:], mag_full[:], AF.Sqrt)          # mag
            nc.scalar.activation(d2[:], d1[:], AF.Exp, scale=-1.0)     # exp(-mag)
            nc.vector.tensor_scalar(mag_full[:], d2[:], 1.0, None, op0=ALU.add)
            nc.scalar.activation(sp_out, mag_full[:], AF.Ln)           # softplus(-mag)
            nc.vector.reciprocal(d2[:], re_ap)                         # 1/re
            nc.vector.tensor_tensor(mag_full[:], im_ap, d2[:], ALU.mult)   # im/re
            nc.scalar.activation(d2[:], mag_full[:], AF.Arctan)        # th0
            nc.vector.tensor_scalar(d1[:], re_ap, 0.0, None, op0=ALU.is_lt)
            nc.scalar.activation(mag_full[:], im_ap, AF.Sign)
            nc.vector.tensor_tensor(th_out, d1[:], mag_full[:], ALU.mult)  # adj (tmp)
            nc.vector.tensor_scalar(d1[:], th_out, PI, None, op0=ALU.mult)
            nc.vector.tensor_tensor(th_out, d2[:], d1[:], ALU.add)

            if dbg:
                nc.sync.dma_start(dbg["dbg_y"][:], y_full[:])

            # ================= Phase C: cumsum (fp32, carried) =============
            # Running carry: tot_ps accumulates column sums of processed tiles
            # (partition 0), copied to SBUF each step for the rank-1 carry add.
            ones_col = wt.tile([P, 1], F32, name="ones_col", tag="ones_col")
            nc.vector.memset(ones_col[:], 1.0)
            tot_ps = ps.tile([1, 128], F32, name="tot_ps", tag="ss", bufs=1)
            for t in range(NT):
                cum_ps = ps.tile([P, 128], F32, name=f"cum_{t}", tag="mm", bufs=5)
                nc.tensor.matmul(cum_ps[:], Utri[:], y_full[:, t * P:(t + 1) * P],
                                 start=True, stop=(t == 0))
                if t > 0:
                    nc.tensor.matmul(cum_ps[:], ones_row[:], tot_sb[:],
                                     start=False, stop=True)
                if t < NT - 1:
                    nc.tensor.matmul(tot_ps[:], ones_col[:],
                                     y_full[:, t * P:(t + 1) * P],
                                     start=(t == 0), stop=(t == NT - 2))
                    tot_sb = io.tile([1, 128], F32, name=f"tot_{t}", tag="tot", bufs=2)
                    nc.vector.tensor_copy(tot_sb[:], tot_ps[:])
                nc.vector.tensor_copy(y_full[:, t * P:(t + 1) * P], cum_ps[:])

            if dbg:
                nc.sync.dma_start(dbg["dbg_cum"][:], y_full[:])

            # ================= Phase D: A / Ainv + apply =================
            cum_sp = y_full[:].rearrange("p (t q d) -> p t q d", q=2, d=DH)[:, :, 0, :]
            cum_th = y_full[:].rearrange("p (t q d) -> p t q d", q=2, d=DH)[:, :, 1, :]
            # cos(x) = sin(x + pi/2); reduce the *shifted* angle into [-pi, pi]
            nc.vector.tensor_scalar(d1[:], cum_th, 1.0 / (2 * PI), 0.25,
                                    op0=ALU.mult, op1=ALU.add)
            nc.vector.tensor_copy(d2[:].bitcast(I32), d1[:])           # round to nearest
            nc.vector.tensor_copy(d1[:], d2[:].bitcast(I32))
            nc.vector.tensor_scalar(d2[:], d1[:], -2 * PI, PI / 2,
                                    op0=ALU.mult, op1=ALU.add)
            nc.vector.tensor_tensor(d1[:], cum_th, d2[:], ALU.add)     # sin arg in [-pi,pi]
            nc.scalar.activation(d2[:], d1[:], AF.Sin)                 # cos
            nc.scalar.activation(d1[:], cum_sp, AF.Exp, scale=-1.0)    # exp(-cum_sp)
            nc.vector.tensor_tensor(A_full[:], d1[:], d2[:], ALU.mult)
            nc.vector.tensor_scalar(d1[:], A_full[:], 1e-10, None, op0=ALU.max)
            Ainv_full = d2
            nc.vector.reciprocal(Ainv_full[:], d1[:])

            if dbg:
                nc.sync.dma_start(dbg["dbg_A"][:], A_full[:])
                nc.sync.dma_start(dbg["dbg_Ainv"][:], Ainv_full[:])

            # pack per tile [A_eff | Ainv_eff], transpose, apply to qkT
            for t in range(NT):
                ab = io.tile([P, 128], F32, name=f"ab_{t}", tag="ab", bufs=3)
                s_t = s_all[:, t:t + 1]
                nc.vector.tensor_scalar(ab[:, 0:DH], A_full[:, t * DH:(t + 1) * DH],
                                        s_t, None, op0=ALU.mult)
                nc.vector.tensor_scalar(ab[:, DH:128], Ainv_full[:, t * DH:(t + 1) * DH],
                                        s_t, None, op0=ALU.mult)
                tp = ps.tile([P, 128], F32, name=f"tp_{t}", tag="mm", bufs=5)
                nc.tensor.transpose(tp[:], ab[:], ident[:])
                nc.vector.tensor_copy(a_sc[:, t * P:(t + 1) * P], tp[:])  # ABT store

            abT = a_sc
            nc.vector.tensor_tensor(qT_eff[:], qkT[0:DH, :], abT[0:DH, :], ALU.mult)
            nc.vector.tensor_tensor(kT_eff[:], qkT[DH:P, :], abT[DH:P, :], ALU.mult)

            if dbg:
                nc.sync.dma_start(dbg["dbg_qT"][:], qT_eff[:].bitcast(F32))
                nc.sync.dma_start(dbg["dbg_kT"][:], kT_eff[:].bitcast(F32))
                nc.sync.dma_start(dbg["dbg_v"][:], v_all[:].bitcast(F32))

            # ================= Phase E: causal attention =================
            for p in range(NPAN):
                ot_ps = ps.tile([DH, PW], F32, name=f"ot_{p}", tag="ot", bufs=1)
                njt = 4 * p + 4
                for j in range(njt):
                    s_ps = ps.tile([P, PW], F32, name=f"s_{p}_{j}", tag="mm", bufs=5)
                    nc.tensor.matmul(s_ps[:], kT_eff[:, j * P:(j + 1) * P],
                                     qT_eff[:, p * PW:(p + 1) * PW],
                                     start=True, stop=True)
                    st_sb = io.tile([P, PW], F32R, name=f"st_{p}_{j}", tag="st", bufs=3)
                    if j // 4 == p:
                        nc.vector.tensor_tensor(st_sb[:], s_ps[:], maskP[j % 4][:],
                                                ALU.mult)
                    elif j % 2 == 0:
                        nc.scalar.copy(st_sb[:], s_ps[:])
                    else:
                        nc.vector.tensor_copy(st_sb[:], s_ps[:])
                    nc.tensor.matmul(ot_ps[:], v_all[:, j * DH:(j + 1) * DH],
                                     st_sb[:], start=(j == 0), stop=(j == njt - 1))
                nc.scalar.copy(y_full[0:DH, p * PW:(p + 1) * PW], ot_ps[:])

            if dbg:
                nc.sync.dma_start(dbg["dbg_outT"][:], y_full[0:DH, :])

            # ================= Phase F: gather + to_out =================
            nc.sync.dma_start(cc_in[:], y_full[0:DH, :])
            nc.gpsimd.collective_compute(
                "AllGather", ALU.bypass, replica_groups=[list(range(8))],
                ins=[cc_in.opt()], outs=[cc_out.opt()])
            for p in range(NPAN):
                f_ps = ps.tile([P, PW], F32, name=f"f_{p}", tag="mm", bufs=5)
                for c in range(4):
                    gc = io.tile([P, PW], F32R, name=f"gc_{p}_{c}", tag="gc", bufs=3)
                    nc.gpsimd.dma_start(gc[:], cc_out[c * P:(c + 1) * P,
                                                      p * PW:(p + 1) * PW])
                    nc.tensor.matmul(f_ps[:], Wo_r[c][:], gc[:],
                                     start=(c == 0), stop=(c == 3))
                of = io.tile([P, PW], F32, name=f"of_{p}", tag="of", bufs=2)
                nc.vector.tensor_scalar(of[:], f_ps[:], bo[:, 0:1], None, op0=ALU.add)
                nc.sync.dma_start(dout[:, p * PW:(p + 1) * PW], of[:])

    nc.compile()
    return nc


def _prep_in_maps(inputs):
    x = np.asarray(inputs["x"], np.float32)[0]            # [4096, 1024]
    gamma = np.asarray(inputs["gamma"], np.float32)
    W_qkv = np.asarray(inputs["W_qkv"], np.float32)
    W_a = np.asarray(inputs["W_a"], np.float32)
    W_out = np.asarray(inputs["W_out"], np.float32)
    b_out = np.asarray(inputs["b_out"], np.float32)

    xT = np.ascontiguousarray(x.T)                        # [1024, 4096]
    ident = np.eye(P, dtype=np.float32)
    Utri = np.triu(np.ones((P, P), np.float32))
    maskP = np.zeros((4, P, PW), np.float32)
    for m in range(4):
        kr = np.arange(P)[:, None]
        qc = np.arange(PW)[None, :]
        maskP[m] = (qc >= m * P + kr).astype(np.float32)
    maskP = maskP.reshape(4 * P, PW)

    g = gamma[:, None]
    in_maps = []
    for h in range(HEADS):
        Wq = g * W_qkv[:, h * DH:(h + 1) * DH] * np.float32(SCALE)
        Wk = g * W_qkv[:, DI + h * DH:DI + (h + 1) * DH]
        Wv = g * W_qkv[:, 2 * DI + h * DH:2 * DI + (h + 1) * DH]
        Wqk = np.ascontiguousarray(np.concatenate([Wq, Wk], 1), np.float32)
        Wa = np.ascontiguousarray(g * W_a[:, h * 128:(h + 1) * 128], np.float32)
        Wo = np.ascontiguousarray(W_out[:, h * 128:(h + 1) * 128], np.float32)
        bo = np.ascontiguousarray(b_out[h * 128:(h + 1) * 128, None], np.float32)
        in_maps.append({
            "xT": xT, "Wqk": Wqk, "Wv": np.ascontiguousarray(Wv), "Wa": Wa,
            "Wo": Wo, "bo": bo, "ident": ident, "Utri": Utri, "maskP": maskP,
        })
    return in_maps


def kernel(**inputs) -> np.ndarray:
    if "nc" not in _cache:
        _cache["nc"] = _build()
    nc = _cache["nc"]
    in_maps = _prep_in_maps(inputs)
    res = run_bass_kernel_spmd(nc, in_maps, core_ids=list(range(8)),
                               **_cache.get("run_kwargs", {}))
    _cache["last_results"] = res
    outT = np.concatenate([res.results[h]["out"] for h in range(HEADS)], axis=0)
    return np.ascontiguousarray(outT.T).reshape(1, SEQ, DIM).astype(np.float32)


# revision 9
# speedup vs baseline: 1.0409x; 1.0409x over previous
"""Trainium2 Bass kernel for nn_CausalFullAttention (8 NeuronCores, SPMD).

Sharding: head-parallel — core h owns head h end-to-end (projections, decay
scan, causal attention), then an on-device AllGather of the per-head output
(transposed layout) lets every core compute a 128-column slice of the final
to_out projection. Host only slices weights / transposes x (layout prep) and
concatenates the 8 output slices.

Numerics: f32r (11-bit mantissa fast matmul) for qkv projections, attention
and to_out; full fp32 for the a-projection and the cumsum tri-matmuls (the
decay scan amplifies rounding); bf16 square trick for the RMS norm row sums.
Host-emulated end-to-end error vs the fp32 reference: ~3.7e-4 Frobenius.
"""
import sys

for _p in ("/opt/trn_rl_repo", "/opt/pypackages"):
    if _p not in sys.path:
        sys.path.append(_p)

import numpy as np
import concourse.bass as bass
import concourse.mybir as mybir
from concourse import bacc, tile
from concourse.bass_utils import run_bass_kernel_spmd

F32 = mybir.dt.float32
F32R = mybir.dt.float32r
BF16 = mybir.dt.bfloat16
I32 = mybir.dt.int32
AF = mybir.ActivationFunctionType
ALU = mybir.AluOpType

HEADS = 8
DH = 64
SEQ = 4096
DIM = 1024
DI = 512               # DIM_INNER
SCALE = DH ** -0.5
P = 128
NT = SEQ // P          # 32 row tiles
NPAN = 8               # q/row panels of 512
PW = 512               # panel width (rows)
NC_ = DIM // P         # 8 contraction chunks
PI = float(np.pi)

_cache = {}


def _build():
    nc = bacc.Bacc("TRN2", target_bir_lowering=False, debug=False,
                   enable_asserts=True, num_devices=8)

    din = {}
    for name, shp in [("xT", [DIM, SEQ]), ("Wqk", [DIM, 128]), ("Wv", [DIM, DH]),
                      ("Wa", [DIM, 128]), ("Wo", [DI, 128]), ("bo", [P, 1]),
                      ("ident", [P, P]), ("Utri", [P, P]), ("maskP", [4 * P, PW]),
                      ("OneHot", [P, 63]), ("U32s", [32, 32])]:
        din[name] = nc.dram_tensor(name, shp, F32, kind="ExternalInput").ap()
    dout = nc.dram_tensor("out", [P, SEQ], F32, kind="ExternalOutput").ap()
    dbg = {}
    if _cache.get("debug"):
        for nm, shp in [("dbg_qkT", [P, SEQ]), ("dbg_a", [P, SEQ]),
                        ("dbg_y", [P, SEQ]), ("dbg_cum", [P, SEQ]),
                        ("dbg_A", [P, NT * DH]), ("dbg_Ainv", [P, NT * DH]),
                        ("dbg_qT", [DH, SEQ]), ("dbg_kT", [DH, SEQ]),
                        ("dbg_v", [P, NT * DH]), ("dbg_s", [P, NT]),
                        ("dbg_outT", [DH, SEQ])]:
            dbg[nm] = nc.dram_tensor(nm, shp, F32, kind="ExternalOutput").ap()

    with tile.TileContext(nc) as tc:
        with tc.tile_pool(name="wt", bufs=1) as wt, \
             tc.tile_pool(name="big", bufs=1) as bg, \
             tc.tile_pool(name="io", bufs=1) as io, \
             tc.tile_pool(name="ps", bufs=1, space="PSUM") as ps, \
             tc.tile_pool(name="dr", bufs=1, space="DRAM") as dr:

            # ---------------- weights / constants ----------------
            Wqk_r, Wv_r, Wa_f = [], [], []
            for c in range(NC_):
                w1 = wt.tile([P, 128], F32R, name=f"wqk{c}", tag=f"wqk{c}")
                nc.gpsimd.dma_start(w1[:], din["Wqk"][c * P:(c + 1) * P, :])
                Wqk_r.append(w1)
                w2 = wt.tile([P, DH], F32R, name=f"wv{c}", tag=f"wv{c}")
                nc.gpsimd.dma_start(w2[:], din["Wv"][c * P:(c + 1) * P, :])
                Wv_r.append(w2)
                w3 = wt.tile([P, 128], F32, name=f"wa{c}", tag=f"wa{c}")
                nc.sync.dma_start(w3[:], din["Wa"][c * P:(c + 1) * P, :])
                Wa_f.append(w3)
            Wo_r = []
            for c in range(4):
                w4 = wt.tile([P, 128], F32R, name=f"wo{c}", tag=f"wo{c}")
                nc.gpsimd.dma_start(w4[:], din["Wo"][c * P:(c + 1) * P, :])
                Wo_r.append(w4)
            bo = wt.tile([P, 1], F32, name="bo", tag="bo")
            nc.sync.dma_start(bo[:], din["bo"][:])
            ident = wt.tile([P, P], F32, name="ident", tag="ident")
            nc.sync.dma_start(ident[:], din["ident"][:])
            Utri = wt.tile([P, P], F32, name="Utri", tag="Utri")
            nc.sync.dma_start(Utri[:], din["Utri"][:])
            maskP = []
            for m in range(4):
                mk = wt.tile([P, PW], F32, name=f"maskP{m}", tag=f"maskP{m}")
                nc.sync.dma_start(mk[:], din["maskP"][m * P:(m + 1) * P, :])
                maskP.append(mk)
            OneHot = wt.tile([P, 63], F32, name="OneHot", tag="OneHot")
            nc.sync.dma_start(OneHot[:], din["OneHot"][:])
            U32s = wt.tile([32, 32], F32, name="U32s", tag="U32s")
            nc.sync.dma_start(U32s[:], din["U32s"][:])
            ones_row = wt.tile([1, P], F32, name="ones_row", tag="ones_row")
            nc.vector.memset(ones_row[:], 1.0)
            ones_bf = wt.tile([P, 1], BF16, name="ones_bf", tag="ones_bf")
            nc.vector.memset(ones_bf[:], 1.0)
            halfpi = wt.tile([P, 1], F32, name="halfpi", tag="halfpi")
            nc.vector.memset(halfpi[:], PI / 2)

            # ---------------- persistent big tensors ----------------
            qkT = bg.tile([P, SEQ], F32, name="qkT", tag="qkT")         # q rows 0-63, k 64-127
            v_all = bg.tile([P, NT * DH], F32R, name="v_all", tag="v_all")
            a_sc = bg.tile([P, SEQ], F32, name="a_sc", tag="a_sc")      # a (scaled); later reused for ABT
            y_full = bg.tile([P, SEQ], F32, name="y_full", tag="y_full")  # [sp|th] per tile; later cum
            mag_full = bg.tile([P, NT * DH], F32, name="mag_full", tag="mag_full")
            d1 = bg.tile([P, NT * DH], F32, name="d1", tag="d1")
            d2 = bg.tile([P, NT * DH], F32, name="d2", tag="d2")
            A_full = bg.tile([P, NT * DH], F32, name="A_full", tag="A_full")
            qT_eff = bg.tile([DH, SEQ], F32R, name="qT_eff", tag="qT_eff")
            kT_eff = bg.tile([DH, SEQ], F32R, name="kT_eff", tag="kT_eff")
            sT_all = bg.tile([P, NT], F32, name="sT_all", tag="sT_all")
            nrm_all = bg.tile([P, NT], F32, name="nrm_all", tag="nrm_all")
            s_all = bg.tile([P, NT], F32, name="s_all", tag="s_all")

            scratch = dr.tile([NPAN, PW], F32, name="scratch", tag="scratch")
            cc_in = dr.tile([DH, SEQ], F32, name="cc_in", tag="cc_in")
            cc_out = dr.tile([DI, SEQ], F32, name="cc_out", tag="cc_out",
                             addr_space="Shared")

            # ================= Phase A: projections + norm =================
            for p in range(NPAN):
                xt, xtr = [], []
                for c in range(NC_):
                    t_ = io.tile([P, PW], F32, name=f"xt_{p}_{c}", tag="xt", bufs=9)
                    nc.sync.dma_start(t_[:], din["xT"][c * P:(c + 1) * P,
                                                       p * PW:(p + 1) * PW])
                    xt.append(t_)
                    tr = io.tile([P, PW], F32R, name=f"xtr_{p}_{c}", tag="xtr", bufs=9)
                    nc.gpsimd.dma_start(tr[:], t_[:])
                    xtr.append(tr)

                # norm row sums via bf16 squares
                ss_ps = ps.tile([1, PW], F32, name=f"ss_{p}", tag="ss", bufs=1)
                for c in range(NC_):
                    sq = io.tile([P, PW], BF16, name=f"sq_{p}_{c}", tag="sq", bufs=2)
                    if c % 2 == 0:
                        nc.scalar.activation(sq[:], xt[c][:], AF.Square)
                    else:
                        nc.vector.tensor_tensor(sq[:], xt[c][:], xt[c][:], ALU.mult)
                    nc.tensor.matmul(ss_ps[:], ones_bf[:], sq[:],
                                     start=(c == 0), stop=(c == NC_ - 1))
                ss_sb = io.tile([1, PW], F32, name=f"ssb_{p}", tag="ssb", bufs=1)
                nc.vector.tensor_copy(ss_sb[:], ss_ps[:])
                nc.sync.dma_start(scratch[p:p + 1, :], ss_sb[:])
                sT_p = sT_all[:, p * 4:(p + 1) * 4]
                nc.sync.dma_start(
                    sT_p, scratch[p:p + 1, :].rearrange("o (t r) -> r (o t)", t=4))
                nrm_p = nrm_all[:, p * 4:(p + 1) * 4]
                nc.scalar.activation(nrm_p, sT_p, AF.Sqrt)
                rcp_p = s_all[:, p * 4:(p + 1) * 4]
                nc.vector.reciprocal(rcp_p, nrm_p)
                nc.vector.tensor_scalar(rcp_p, rcp_p, 32.0, None, op0=ALU.mult)

                # qkT (f32r)
                qk_ps = ps.tile([P, PW], F32, name=f"qk_{p}", tag="mm", bufs=5)
                for c in range(NC_):
                    nc.tensor.matmul(qk_ps[:], Wqk_r[c][:], xtr[c][:],
                                     start=(c == 0), stop=(c == NC_ - 1))
                nc.scalar.copy(qkT[:, p * PW:(p + 1) * PW], qk_ps[:])

                # vT (f32r) then transpose to row layout with s-scale
                v_ps = ps.tile([DH, PW], F32, name=f"v_{p}", tag="mm", bufs=5)
                for c in range(NC_):
                    nc.tensor.matmul(v_ps[:], Wv_r[c][:], xtr[c][:],
                                     start=(c == 0), stop=(c == NC_ - 1))
                vT_sb = io.tile([DH, PW], F32, name=f"vts_{p}", tag="vt", bufs=2)
                nc.scalar.copy(vT_sb[:], v_ps[:])
                for tt in range(4):
                    g = p * 4 + tt
                    vp2 = ps.tile([P, DH], F32, name=f"vp2_{p}_{tt}", tag="mm", bufs=5)
                    nc.tensor.transpose(vp2[:], vT_sb[:, tt * P:(tt + 1) * P],
                                        ident[0:DH, 0:DH])
                    nc.vector.tensor_scalar(v_all[:, g * DH:(g + 1) * DH], vp2[:],
                                            s_all[:, g:g + 1], None, op0=ALU.mult)

                # a projection (fp32, transposed) + transpose back + row scale
                aT_ps = ps.tile([P, PW], F32, name=f"aT_{p}", tag="mm", bufs=5)
                for c in range(NC_):
                    nc.tensor.matmul(aT_ps[:], Wa_f[c][:], xt[c][:],
                                     start=(c == 0), stop=(c == NC_ - 1))
                aT_sb = io.tile([P, PW], F32, name=f"aTs_{p}", tag="at", bufs=1)
                nc.scalar.copy(aT_sb[:], aT_ps[:])
                for tt in range(4):
                    g = p * 4 + tt
                    a_tp = ps.tile([P, P], F32, name=f"atp_{p}_{tt}", tag="mm", bufs=5)
                    nc.tensor.transpose(a_tp[:], aT_sb[:, tt * P:(tt + 1) * P],
                                        ident[:])
                    nc.vector.tensor_scalar(a_sc[:, g * P:(g + 1) * P], a_tp[:],
                                            s_all[:, g:g + 1], None, op0=ALU.mult)

            if dbg:
                nc.sync.dma_start(dbg["dbg_qkT"][:], qkT[:])
                nc.sync.dma_start(dbg["dbg_a"][:], a_sc[:])
                nc.sync.dma_start(dbg["dbg_s"][:], s_all[:])

            # ================= Phase B: decay elementwise =================
            # a_sc per tile: cols g*128+(2d+c); re: c=0, im: c=1
            re_ap = a_sc[:].rearrange("p (t d c) -> p (t d) c", c=2, d=DH)[:, :, 0]
            im_ap = a_sc[:].rearrange("p (t d c) -> p (t d) c", c=2, d=DH)[:, :, 1]
            sp_out = y_full[:].rearrange("p (t q d) -> p t q d", q=2, d=DH)[:, :, 0, :]
            th_out = y_full[:].rearrange("p (t q d) -> p t q d", q=2, d=DH)[:, :, 1, :]
            nc.vector.tensor_tensor(d1[:], re_ap, re_ap, ALU.mult)
            nc.vector.tensor_tensor(d2[:], im_ap, im_ap, ALU.mult)
            nc.vector.tensor_tensor(mag_full[:], d1[:], d2[:], ALU.add)
            nc.scalar.activation(d1[:], mag_full[:], AF.Sqrt)          # mag
            nc.scalar.activation(d2[:], d1[:], AF.Exp, scale=-1.0)     # exp(-mag)
            nc.vector.tensor_scalar(mag_full[:], d2[:], 1.0, None, op0=ALU.add)
            nc.scalar.activation(sp_out, mag_full[:], AF.Ln)           # softplus(-mag)
            nc.vector.reciprocal_approx_accurate(d2[:], re_ap, mag_full[:])  # 1/re
            nc.vector.tensor_tensor(mag_full[:], im_ap, d2[:], ALU.mult)   # im/re
            nc.scalar.activation(d2[:], mag_full[:], AF.Arctan)        # th0
            nc.vector.tensor_scalar(d1[:], re_ap, 0.0, None, op0=ALU.is_lt)
            nc.scalar.activation(mag_full[:], im_ap, AF.Sign)
            nc.vector.tensor_tensor(th_out, d1[:], mag_full[:], ALU.mult)  # adj (tmp)
            nc.vector.tensor_scalar(d1[:], th_out, PI, None, op0=ALU.mult)
            nc.vector.tensor_tensor(th_out, d2[:], d1[:], ALU.add)

            if dbg:
                nc.sync.dma_start(dbg["dbg_y"][:], y_full[:])

            # ================= Phase C: cumsum (fp32, 3 matmul generations) ==
            # (a) per-tile column sums packed to psum rows via one-hot lhsT
            tot32_ps = ps.tile([32, 128], F32, name="tot32", tag="ss", bufs=1)
            for t in range(NT):
                nc.tensor.matmul(tot32_ps[:], OneHot[:, 31 - t:63 - t],
                                 y_full[:, t * P:(t + 1) * P],
                                 start=(t == 0), stop=(t == NT - 1))
            tot32_sb = io.tile([32, 128], F32, name="tot32_sb", tag="tot", bufs=1)
            nc.vector.tensor_copy(tot32_sb[:], tot32_ps[:])
            # (b) exclusive prefix over tile sums
            carr_ps = ps.tile([32, 128], F32, name="carr", tag="mm", bufs=5)
            nc.tensor.matmul(carr_ps[:], U32s[:], tot32_sb[:], start=True, stop=True)
            carr_sb = io.tile([32, 128], F32, name="carr_sb", tag="carrs", bufs=1)
            nc.vector.tensor_copy(carr_sb[:], carr_ps[:])
            carr_dram = dr.tile([32, 128], F32, name="carr_dram", tag="carrd")
            nc.sync.dma_start(carr_dram[:], carr_sb[:])
            cts = [None]
            for t in range(1, NT):
                ct = io.tile([1, 128], F32, name=f"ct_{t}", tag="ct", bufs=2)
                nc.sync.dma_start(ct[:], carr_dram[t:t + 1, :])
                cts.append(ct)
            # (c) local cumsum + broadcast carry
            for t in range(NT):
                cum_ps = ps.tile([P, 128], F32, name=f"cum_{t}", tag="mm", bufs=5)
                nc.tensor.matmul(cum_ps[:], Utri[:], y_full[:, t * P:(t + 1) * P],
                                 start=True, stop=(t == 0))
                if t > 0:
                    nc.tensor.matmul(cum_ps[:], ones_row[:], cts[t][:],
                                     start=False, stop=True)
                nc.vector.tensor_copy(y_full[:, t * P:(t + 1) * P], cum_ps[:])

            if dbg:
                nc.sync.dma_start(dbg["dbg_cum"][:], y_full[:])

            # ================= Phase D: A / Ainv + apply =================
            cum_sp = y_full[:].rearrange("p (t q d) -> p t q d", q=2, d=DH)[:, :, 0, :]
            cum_th = y_full[:].rearrange("p (t q d) -> p t q d", q=2, d=DH)[:, :, 1, :]
            # cos(x) = sin(x + pi/2); reduce the *shifted* angle into [-pi, pi]
            nc.vector.tensor_scalar(d1[:], cum_th, 1.0 / (2 * PI), 0.25,
                                    op0=ALU.mult, op1=ALU.add)
            nc.vector.tensor_copy(d2[:].bitcast(I32), d1[:])           # round to nearest
            nc.vector.tensor_copy(d1[:], d2[:].bitcast(I32))
            nc.vector.tensor_scalar(d2[:], d1[:], -2 * PI, PI / 2,
                                    op0=ALU.mult, op1=ALU.add)
            nc.vector.tensor_tensor(d1[:], cum_th, d2[:], ALU.add)     # sin arg in [-pi,pi]
            nc.scalar.activation(d2[:], d1[:], AF.Sin)                 # cos
            nc.scalar.activation(d1[:], cum_sp, AF.Exp, scale=-1.0)    # exp(-cum_sp)
            nc.vector.tensor_tensor(A_full[:], d1[:], d2[:], ALU.mult)
            nc.vector.tensor_scalar(d1[:], A_full[:], 1e-10, None, op0=ALU.max)
            Ainv_full = d2
            nc.vector.reciprocal_approx_accurate(Ainv_full[:], d1[:], mag_full[:])

            if dbg:
                nc.sync.dma_start(dbg["dbg_A"][:], A_full[:])
                nc.sync.dma_start(dbg["dbg_Ainv"][:], Ainv_full[:])

            # pack per tile [A_eff | Ainv_eff], transpose, apply to qkT
            for t in range(NT):
                ab = io.tile([P, 128], F32, name=f"ab_{t}", tag="ab", bufs=3)
                s_t = s_all[:, t:t + 1]
                nc.vector.tensor_scalar(ab[:, 0:DH], A_full[:, t * DH:(t + 1) * DH],
                                        s_t, None, op0=ALU.mult)
                nc.vector.tensor_scalar(ab[:, DH:128], Ainv_full[:, t * DH:(t + 1) * DH],
                                        s_t, None, op0=ALU.mult)
                tp = ps.tile([P, 128], F32, name=f"tp_{t}", tag="mm", bufs=5)
                nc.tensor.transpose(tp[:], ab[:], ident[:])
                nc.vector.tensor_copy(a_sc[:, t * P:(t + 1) * P], tp[:])  # ABT store

            abT = a_sc
            nc.vector.tensor_tensor(qT_eff[:], qkT[0:DH, :], abT[0:DH, :], ALU.mult)
            nc.vector.tensor_tensor(kT_eff[:], qkT[DH:P, :], abT[DH:P, :], ALU.mult)

            if dbg:
                nc.sync.dma_start(dbg["dbg_qT"][:], qT_eff[:].bitcast(F32))
                nc.sync.dma_start(dbg["dbg_kT"][:], kT_eff[:].bitcast(F32))
                nc.sync.dma_start(dbg["dbg_v"][:], v_all[:].bitcast(F32))

            # ================= Phase E: causal attention =================
            for p in range(NPAN):
                ot_ps = ps.tile([DH, PW], F32, name=f"ot_{p}", tag="ot", bufs=1)
                njt = 4 * p + 4
                for j in range(njt):
                    s_ps = ps.tile([P, PW], F32, name=f"s_{p}_{j}", tag="mm", bufs=5)
                    nc.tensor.matmul(s_ps[:], kT_eff[:, j * P:(j + 1) * P],
                                     qT_eff[:, p * PW:(p + 1) * PW],
                                     start=True, stop=True)
                    st_sb = io.tile([P, PW], F32R, name=f"st_{p}_{j}", tag="st", bufs=3)
                    if j // 4 == p:
                        nc.vector.tensor_tensor(st_sb[:], s_ps[:], maskP[j % 4][:],
                                                ALU.mult)
                    elif j % 3 != 0:
                        nc.scalar.copy(st_sb[:], s_ps[:])
                    else:
                        nc.vector.tensor_copy(st_sb[:], s_ps[:])
                    nc.tensor.matmul(ot_ps[:], v_all[:, j * DH:(j + 1) * DH],
                                     st_sb[:], start=(j == 0), stop=(j == njt - 1))
                nc.scalar.copy(y_full[0:DH, p * PW:(p + 1) * PW], ot_ps[:])

            if dbg:
                nc.sync.dma_start(dbg["dbg_outT"][:], y_full[0:DH, :])

            # ================= Phase F: gather + to_out =================
            nc.sync.dma_start(cc_in[:], y_full[0:DH, :])
            nc.gpsimd.collective_compute(
                "AllGather", ALU.bypass, replica_groups=[list(range(8))],
                ins=[cc_in.opt()], outs=[cc_out.opt()])
            for p in range(NPAN):
                f_ps = ps.tile([P, PW], F32, name=f"f_{p}", tag="mm", bufs=5)
                for c in range(4):
                    gc = io.tile([P, PW], F32R, name=f"gc_{p}_{c}", tag="gc", bufs=2)
                    nc.gpsimd.dma_start(gc[:], cc_out[c * P:(c + 1) * P,
                                                      p * PW:(p + 1) * PW])
                    nc.tensor.matmul(f_ps[:], Wo_r[c][:], gc[:],
                                     start=(c == 0), stop=(c == 3))
                of = io.tile([P, PW], F32, name=f"of_{p}", tag="of", bufs=1)
                nc.vector.tensor_scalar(of[:], f_ps[:], bo[:, 0:1], None, op0=ALU.add)
                nc.sync.dma_start(dout[:, p * PW:(p + 1) * PW], of[:])

    nc.compile()
    return nc


def _prep_in_maps(inputs):
    x = np.asarray(inputs["x"], np.float32)[0]            # [4096, 1024]
    gamma = np.asarray(inputs["gamma"], np.float32)
    W_qkv = np.asarray(inputs["W_qkv"], np.float32)
    W_a = np.asarray(inputs["W_a"], np.float32)
    W_out = np.asarray(inputs["W_out"], np.float32)
    b_out = np.asarray(inputs["b_out"], np.float32)

    xT = np.ascontiguousarray(x.T)                        # [1024, 4096]
    ident = np.eye(P, dtype=np.float32)
    Utri = np.triu(np.ones((P, P), np.float32))
    maskP = np.zeros((4, P, PW), np.float32)
    for m in range(4):
        kr = np.arange(P)[:, None]
        qc = np.arange(PW)[None, :]
        maskP[m] = (qc >= m * P + kr).astype(np.float32)
    maskP = maskP.reshape(4 * P, PW)
    OneHot = np.zeros((P, 63), np.float32)
    OneHot[:, 31] = 1.0
    U32s = np.triu(np.ones((32, 32), np.float32), 1)

    g = gamma[:, None]
    in_maps = []
    for h in range(HEADS):
        Wq = g * W_qkv[:, h * DH:(h + 1) * DH] * np.float32(SCALE)
        Wk = g * W_qkv[:, DI + h * DH:DI + (h + 1) * DH]
        Wv = g * W_qkv[:, 2 * DI + h * DH:2 * DI + (h + 1) * DH]
        Wqk = np.ascontiguousarray(np.concatenate([Wq, Wk], 1), np.float32)
        Wa = np.ascontiguousarray(g * W_a[:, h * 128:(h + 1) * 128], np.float32)
        Wo = np.ascontiguousarray(W_out[:, h * 128:(h + 1) * 128], np.float32)
        bo = np.ascontiguousarray(b_out[h * 128:(h + 1) * 128, None], np.float32)
        in_maps.append({
            "xT": xT, "Wqk": Wqk, "Wv": np.ascontiguousarray(Wv), "Wa": Wa,
            "Wo": Wo, "bo": bo, "ident": ident, "Utri": Utri, "maskP": maskP,
            "OneHot": OneHot, "U32s": U32s,
        })
    return in_maps


def kernel(**inputs) -> np.ndarray:
    if "nc" not in _cache:
        _cache["nc"] = _build()
    nc = _cache["nc"]
    in_maps = _prep_in_maps(inputs)
    res = run_bass_kernel_spmd(nc, in_maps, core_ids=list(range(8)),
                               **_cache.get("run_kwargs", {}))
    _cache["last_results"] = res
    outT = np.concatenate([res.results[h]["out"] for h in range(HEADS)], axis=0)
    return np.ascontiguousarray(outT.T).reshape(1, SEQ, DIM).astype(np.float32)
